# revision 5
# baseline (speedup 1.0000x reference)
"""Binary complex conv (BC conv) on 8 TRN2 NeuronCores.

Reference computation:
    xb = sign(x + 1e-6)                      # (16, 256, 112, 112)
    wr = sign(weight_real + 1e-6)            # (128, 128, 3, 3)
    wi = sign(weight_imag + 1e-6)
    kernel = [[wr, -wi], [wi, wr]]           # (256, 256, 3, 3)
    out = conv2d(xb, kernel, pad=1) + bias   # (16, 256, 112, 112)

Strategy: pure data-parallel over batch (2 images per core), everything
else on-device. All matmul operands are exactly representable in bf16
(+-1 / 0 / +-2) and PSUM accumulates in fp32, so the result is exact.

Layout: each image lives in SBUF as a zero-padded 114x114 frame per
128-channel block, plus one junk margin row above/below, flattened in
the free dim. A conv tap (dy,dx) is then a single flat offset
dy*114+dx, so each output tile of 4 rows x 114 cols (456 <= 512 PSUM
bank) accumulates 9 taps x 2 ci-blocks with plain strided matmuls.
Pad columns produce garbage lanes that are skipped on eviction.
"""

import numpy as np

import concourse.bass as bass
import concourse.tile as tile
from concourse import mybir
from concourse.bass_utils import run_bass_kernel_spmd

N_CORES = 8
B = 16
CPB = 128          # channels per block (partition dim)
H = W = 112
PW = 114           # padded frame width/height
ROWS = 116         # padded frame rows + 1 margin row top/bottom
IMGS = 2           # images per core
TROWS = 4          # output rows per matmul tile
NTILES = H // TROWS
BAND = 28          # input rows binarized per activation op
EPS = 1e-6

F32 = mybir.dt.float32
BF16 = mybir.dt.bfloat16
AF = mybir.ActivationFunctionType


def _split_multiwait(nc):
    """Walrus in this container rejects >1 semaphore wait per instruction
    ("Too many sync wait commands"); hoist extra waits onto preceding nops
    on the same engine."""
    import bass_rust

    for f in nc.m.functions:
        for bb in f.blocks:
            new_insts = []
            for inst in bb.instructions:
                si = inst.sync_info
                waits = list(si.on_wait) if si is not None and si.on_wait else []
                if len(waits) > 1:
                    for w in waits[:-1]:
                        nop = mybir.InstNoOp(
                            name=nc.get_next_instruction_name(),
                            engine=inst.engine,
                            ins=[],
                            outs=[],
                        )
                        nop.sync_info = bass_rust.SyncInfo(on_wait=[w], on_update=[])
                        new_insts.append(nop)
                    si.on_wait = [waits[-1]]
                    inst.sync_info = si
                new_insts.append(inst)
            bb.instructions = new_insts


def build_nc():
    nc = bass.Bass()

    x_ext = nc.declare_dram_parameter("x", [IMGS, 2 * CPB, H, W], F32, isOutput=False)
    wr_ext = nc.declare_dram_parameter("wrT", [CPB, 9 * CPB], F32, isOutput=False)
    wi_ext = nc.declare_dram_parameter("wiT", [CPB, 9 * CPB], F32, isOutput=False)
    bias_ext = nc.declare_dram_parameter("bias2", [CPB, 2], F32, isOutput=False)
    out_ext = nc.declare_dram_parameter("out", [IMGS, 2 * CPB, H, W], F32, isOutput=True)

    x_flat = x_ext.rearrange("b c h w -> (b c) h w")       # [2*256, 112, 112]
    out_flat = out_ext.rearrange("b c h w -> (b c) h w")

    with tile.TileContext(nc) as tc:
        with (
            tc.tile_pool(name="wstage", bufs=2) as wstage_pool,
            tc.tile_pool(name="wbin", bufs=1) as wbin_pool,
            tc.tile_pool(name="biasp", bufs=1) as bias_pool,
            tc.tile_pool(name="xb", bufs=1) as xb_pool,
            tc.tile_pool(name="stage", bufs=3) as stage_pool,
            tc.tile_pool(name="outsb", bufs=4) as out_pool,
            tc.tile_pool(name="psum", bufs=6, space="PSUM") as psum_pool,
        ):
            # per-partition scalar constants for activation bias (the
            # container's const-AP registry has no entry for 1e-6)
            eps_pos = bias_pool.tile([CPB, 1], F32, tag="epsp")
            eps_neg = bias_pool.tile([CPB, 1], F32, tag="epsn")
            nc.gpsimd.memset(eps_pos[:], EPS)
            nc.gpsimd.memset(eps_neg[:], -EPS)

            # ---- weights: DMA f32, binarize into 3 sign combos (bf16) ----
            wr_f32 = wstage_pool.tile([CPB, 9 * CPB], F32, tag="wstage")
            nc.sync.dma_start(wr_f32[:], wr_ext[:])
            wi_f32 = wstage_pool.tile([CPB, 9 * CPB], F32, tag="wstage")
            nc.sync.dma_start(wi_f32[:], wi_ext[:])

            wbr = wbin_pool.tile([CPB, 9, CPB], BF16, tag="wbr")
            wbi = wbin_pool.tile([CPB, 9, CPB], BF16, tag="wbi")
            wbin = wbin_pool.tile([CPB, 9, CPB], BF16, tag="wbin")
            wr_v = wr_f32[:].rearrange("p (t c) -> p t c", c=CPB)
            wi_v = wi_f32[:].rearrange("p (t c) -> p t c", c=CPB)
            nc.scalar.activation(wbr[:], wr_v, AF.Sign, bias=eps_pos[:], scale=1.0)
            nc.scalar.activation(wbi[:], wi_v, AF.Sign, bias=eps_pos[:], scale=1.0)
            # -sign(x+eps) == sign(-x-eps)
            nc.scalar.activation(wbin[:], wi_v, AF.Sign, bias=eps_neg[:], scale=-1.0)

            bias_sb = bias_pool.tile([CPB, 2], F32)
            nc.sync.dma_start(bias_sb[:], bias_ext[:])

            # weight select: [co_block][ci_block] -> binarized lhsT source
            wsel = [[wbr, wbin], [wbi, wbr]]

            # ---- persistent padded binarized input frames ----
            # xb[img][cib]: [128, ROWS, PW] bf16; tile row = padded row + 1
            xb = [
                [xb_pool.tile([CPB, ROWS, PW], BF16, tag=f"xb{i}{c}",
                              name=f"xb{i}{c}") for c in range(2)]
                for i in range(IMGS)
            ]
            xb_flat = [[xb[i][c][:].rearrange("p r c -> p (r c)") for c in range(2)]
                       for i in range(IMGS)]
            for i in range(IMGS):
                for c in range(2):
                    t = xb[i][c]
                    nc.gpsimd.memset(t[:, 1:2, :], 0.0)        # padded row 0
                    nc.gpsimd.memset(t[:, 114:115, :], 0.0)    # padded row 113
                    nc.gpsimd.memset(t[:, 1:115, 0:1], 0.0)    # padded col 0
                    nc.gpsimd.memset(t[:, 1:115, 113:114], 0.0)  # padded col 113

            # ---- binarize input, band by band ----
            def binarize_image(img):
                for cib in range(2):
                    ch0 = img * 2 * CPB + cib * CPB
                    for b in range(H // BAND):
                        r0 = b * BAND
                        st = stage_pool.tile([CPB, BAND, W], F32, tag="stage")
                        nc.sync.dma_start(
                            st[:], x_flat[ch0:ch0 + CPB, r0:r0 + BAND, :]
                        )
                        nc.scalar.activation(
                            xb[img][cib][:, r0 + 2:r0 + 2 + BAND, 1:113],
                            st[:],
                            AF.Sign,
                            bias=eps_pos[:],
                            scale=1.0,
                        )

            def conv_image(img):
                for t in range(NTILES):
                    base = (4 * t + 2) * PW
                    for coblk in range(2):
                        ps = psum_pool.tile([CPB, TROWS * PW], F32, tag="ps")
                        first = True
                        for cib in range(2):
                            w3 = wsel[coblk][cib]
                            for tap in range(9):
                                dy, dx = tap // 3 - 1, tap % 3 - 1
                                s = base + dy * PW + dx
                                nc.tensor.matmul(
                                    ps[:],
                                    w3[:, tap, :],
                                    xb_flat[img][cib][:, s:s + TROWS * PW],
                                    start=first,
                                    stop=(cib == 1 and tap == 8),
                                )
                                first = False
                        # evict: skip pad columns, add per-channel bias
                        osb = out_pool.tile([CPB, TROWS, W], F32, tag="osb")
                        psv = ps[:].rearrange("p (r c) -> p r c", c=PW)
                        nc.vector.tensor_scalar_add(
                            osb[:], psv[:, :, 1:113], bias_sb[:, coblk:coblk + 1]
                        )
                        ch0 = img * 2 * CPB + coblk * CPB
                        nc.sync.dma_start(
                            out_flat[ch0:ch0 + CPB, 4 * t:4 * t + TROWS, :], osb[:]
                        )

            binarize_image(0)
            conv_image(0)
            binarize_image(1)
            conv_image(1)

    _split_multiwait(nc)
    return nc


def kernel(x, weight_real, weight_imag, bias):
    x = np.ascontiguousarray(np.asarray(x, dtype=np.float32))
    wr = np.asarray(weight_real, dtype=np.float32)
    wi = np.asarray(weight_imag, dtype=np.float32)
    bias = np.asarray(bias, dtype=np.float32)

    # host-side pure layout prep: lhsT layout [ci, tap, co]
    wrT = np.ascontiguousarray(
        wr.transpose(1, 2, 3, 0).reshape(CPB, 9 * CPB)
    )
    wiT = np.ascontiguousarray(
        wi.transpose(1, 2, 3, 0).reshape(CPB, 9 * CPB)
    )
    bias2 = np.ascontiguousarray(bias.reshape(2, CPB).T)  # [128, 2]

    nc = build_nc()
    in_maps = [
        {
            "x": x[IMGS * c:IMGS * (c + 1)],
            "wrT": wrT,
            "wiT": wiT,
            "bias2": bias2,
        }
        for c in range(N_CORES)
    ]
    res = run_bass_kernel_spmd(nc, in_maps, core_ids=list(range(N_CORES)))
    return np.concatenate([res.results[i]["out"] for i in range(N_CORES)], axis=0)


def run_traced(x, weight_real, weight_imag, bias, **trace_kwargs):
    """test.py entry: same as kernel() but with neuron-profile tracing."""
    x = np.ascontiguousarray(np.asarray(x, dtype=np.float32))
    wr = np.asarray(weight_real, dtype=np.float32)
    wi = np.asarray(weight_imag, dtype=np.float32)
    bias = np.asarray(bias, dtype=np.float32)
    wrT = np.ascontiguousarray(wr.transpose(1, 2, 3, 0).reshape(CPB, 9 * CPB))
    wiT = np.ascontiguousarray(wi.transpose(1, 2, 3, 0).reshape(CPB, 9 * CPB))
    bias2 = np.ascontiguousarray(bias.reshape(2, CPB).T)

    nc = build_nc()
    in_maps = [
        {"x": x[IMGS * c:IMGS * (c + 1)], "wrT": wrT, "wiT": wiT, "bias2": bias2}
        for c in range(N_CORES)
    ]
    res = run_bass_kernel_spmd(
        nc, in_maps, core_ids=list(range(N_CORES)), trace=True, **trace_kwargs
    )
    out = np.concatenate([res.results[i]["out"] for i in range(N_CORES)], axis=0)
    return out, res


# revision 7
# speedup vs baseline: 1.8081x; 1.8081x over previous
"""Binary complex conv (BC conv) on 8 TRN2 NeuronCores.

Reference computation:
    xb = sign(x + 1e-6)                      # (16, 256, 112, 112)
    wr = sign(weight_real + 1e-6)            # (128, 128, 3, 3)
    wi = sign(weight_imag + 1e-6)
    kernel = [[wr, -wi], [wi, wr]]           # (256, 256, 3, 3)
    out = conv2d(xb, kernel, pad=1) + bias   # (16, 256, 112, 112)

Strategy: pure data-parallel over batch (2 images per core); everything
else on-device, numerically exact (all matmul operands are +-1/0/+-2 ->
exact in fp8e4/bf16; PSUM accumulates fp32).

Two tricks on top of the direct conv:
 * Karatsuba for the complex structure: A = xr*wr, B = xi*wi,
   C = (xr+xi)*(wr+wi); out_real = A-B, out_imag = C-A-B.
   3 convs of 128 input channels instead of 4.
 * fp8 DoubleRow: each binarized frame is stored with row stride 116;
   conv taps in raster order have flat offsets [-117,-116,-115,-1,0,1,
   115,116,117], so consecutive taps pair into DoubleRow matmuls
   (contraction 256) with pair strides 1/114/1/1 + one normal matmul.

Each 4-output-row tile accumulates into a [128, 464] PSUM bank
(garbage pad lanes skipped on eviction).
"""

import numpy as np

import concourse.bass as bass
import concourse.tile as tile
from concourse import mybir
from concourse.bass_utils import run_bass_kernel_spmd

N_CORES = 8
B = 16
CPB = 128          # channels per block (partition dim)
H = W = 112
RS = 116           # frame row stride
FROWS = 116        # 114 padded rows + 2 junk margin rows (13456 B, %16==0)
IMGS = 2
TROWS = 4          # output rows per matmul tile
NT = TROWS * RS    # matmul free dim (464)
NTILES = H // TROWS
BAND = 28          # input rows binarized per activation op
EPS = 1e-6

F32 = mybir.dt.float32
FP8 = mybir.dt.float8e4
AF = mybir.ActivationFunctionType
DRM = mybir.MatmulPerfMode.DoubleRow
ALU = mybir.AluOpType

# tap flat offsets in raster order; pairs (0,1) (2,3) (4,5) (6,7), single 8
TAP_OFF = [dy * RS + dx for dy in (-1, 0, 1) for dx in (-1, 0, 1)]


def _split_multiwait(nc):
    """Walrus in this container rejects >1 semaphore wait per instruction
    ("Too many sync wait commands"); hoist extra waits onto preceding nops
    on the same engine."""
    import bass_rust

    for f in nc.m.functions:
        for bb in f.blocks:
            new_insts = []
            for inst in bb.instructions:
                si = inst.sync_info
                waits = list(si.on_wait) if si is not None and si.on_wait else []
                if len(waits) > 1:
                    for w in waits[:-1]:
                        nop = mybir.InstNoOp(
                            name=nc.get_next_instruction_name(),
                            engine=inst.engine,
                            ins=[],
                            outs=[],
                        )
                        nop.sync_info = bass_rust.SyncInfo(on_wait=[w], on_update=[])
                        new_insts.append(nop)
                    si.on_wait = [waits[-1]]
                    inst.sync_info = si
                new_insts.append(inst)
            bb.instructions = new_insts


def build_nc():
    nc = bass.Bass()

    x_ext = nc.declare_dram_parameter("x", [IMGS, 2 * CPB, H, W], F32, isOutput=False)
    wr_ext = nc.declare_dram_parameter("wrT", [CPB, 9 * CPB], F32, isOutput=False)
    wi_ext = nc.declare_dram_parameter("wiT", [CPB, 9 * CPB], F32, isOutput=False)
    bias_ext = nc.declare_dram_parameter("bias2", [CPB, 2], F32, isOutput=False)
    out_ext = nc.declare_dram_parameter("out", [IMGS, 2 * CPB, H, W], F32, isOutput=True)

    x_flat = x_ext.rearrange("b c h w -> (b c) h w")       # [512, 112, 112]
    out_flat = out_ext.rearrange("b c h w -> (b c) h w")

    with tile.TileContext(nc) as tc:
        with (
            tc.tile_pool(name="wstage", bufs=2) as wstage_pool,
            tc.tile_pool(name="wbin", bufs=1) as wbin_pool,
            tc.tile_pool(name="biasp", bufs=1) as bias_pool,
            tc.tile_pool(name="xq", bufs=1) as xq_pool,
            tc.tile_pool(name="stage", bufs=3) as stage_pool,
            tc.tile_pool(name="tmp", bufs=4) as tmp_pool,
            tc.tile_pool(name="outsb", bufs=4) as out_pool,
            tc.tile_pool(name="psum", bufs=6, space="PSUM") as psum_pool,
        ):
            # per-partition scalar constant for activation bias
            eps_pos = bias_pool.tile([CPB, 1], F32, tag="epsp")
            nc.gpsimd.memset(eps_pos[:], EPS)

            # ---- weights ----
            wr_f32 = wstage_pool.tile([CPB, 9 * CPB], F32, tag="wstage")
            nc.sync.dma_start(wr_f32[:], wr_ext[:])
            wi_f32 = wstage_pool.tile([CPB, 9 * CPB], F32, tag="wstage")
            nc.sync.dma_start(wi_f32[:], wi_ext[:])

            # binarized fp8 weights [ci, tap, co] + their f32 forms for the sum
            wq_r = wbin_pool.tile([CPB, 9, CPB], FP8, tag="wqr")
            wq_i = wbin_pool.tile([CPB, 9, CPB], FP8, tag="wqi")
            wq_s = wbin_pool.tile([CPB, 9, CPB], FP8, tag="wqs")
            w32_r = wstage_pool.tile([CPB, 9 * CPB], F32, tag="w32r")
            w32_i = wstage_pool.tile([CPB, 9 * CPB], F32, tag="w32i")
            w32_s = wstage_pool.tile([CPB, 9 * CPB], F32, tag="w32s")
            wr_v = wr_f32[:].rearrange("p (t c) -> p t c", c=CPB)
            wi_v = wi_f32[:].rearrange("p (t c) -> p t c", c=CPB)
            nc.scalar.activation(wq_r[:], wr_v, AF.Sign, bias=eps_pos[:], scale=1.0)
            nc.scalar.activation(wq_i[:], wi_v, AF.Sign, bias=eps_pos[:], scale=1.0)
            nc.scalar.activation(w32_r[:], wr_f32[:], AF.Sign, bias=eps_pos[:], scale=1.0)
            nc.scalar.activation(w32_i[:], wi_f32[:], AF.Sign, bias=eps_pos[:], scale=1.0)
            nc.vector.tensor_tensor(w32_s[:], w32_r[:], w32_i[:], op=ALU.add)
            nc.scalar.activation(
                wq_s[:], w32_s[:].rearrange("p (t c) -> p t c", c=CPB), AF.Copy
            )

            bias_sb = bias_pool.tile([CPB, 2], F32)
            nc.sync.dma_start(bias_sb[:], bias_ext[:])
            bias_d = bias_pool.tile([CPB, 1], F32, tag="biasd")
            nc.vector.tensor_sub(bias_d[:], bias_sb[:, 1:2], bias_sb[:, 0:1])

            # ---- persistent binarized fp8 frames ----
            # frame: [128, FROWS, RS]; frame row = padded row + 1 (1 junk
            # margin row on top); cols 0 / 113 are the zero pad columns,
            # cols 114-115 slack (only ever read into discarded pad lanes)
            def frame(nm):
                return xq_pool.tile([CPB, FROWS, RS], FP8, tag=nm, name=nm)

            xqr = [frame(f"xqr{i}") for i in range(IMGS)]
            xqi = [frame(f"xqi{i}") for i in range(IMGS)]
            xqs = [frame(f"xqs{i}") for i in range(IMGS)]
            for frames in (xqr, xqi, xqs):
                for t in frames:
                    nc.gpsimd.memset(t[:, 1:2, :], 0.0)          # padded row 0
                    nc.gpsimd.memset(t[:, 114:115, :], 0.0)      # padded row 113
                    nc.gpsimd.memset(t[:, 1:115, 0:1], 0.0)      # padded col 0
                    nc.gpsimd.memset(t[:, 1:115, 113:114], 0.0)  # padded col 113

            flat = {}
            for i in range(IMGS):
                flat[("r", i)] = xqr[i][:].rearrange("p r c -> p (r c)")
                flat[("i", i)] = xqi[i][:].rearrange("p r c -> p (r c)")
                flat[("s", i)] = xqs[i][:].rearrange("p r c -> p (r c)")

            # ---- binarize input + build the sum frame, band by band ----
            def binarize_image(img):
                for b in range(H // BAND):
                    r0 = b * BAND
                    rows = slice(r0 + 2, r0 + 2 + BAND)
                    for cib, dst in ((0, xqr), (1, xqi)):
                        ch0 = img * 2 * CPB + cib * CPB
                        st = stage_pool.tile([CPB, BAND, W], F32, tag="stage")
                        nc.sync.dma_start(
                            st[:], x_flat[ch0:ch0 + CPB, r0:r0 + BAND, :]
                        )
                        nc.scalar.activation(
                            dst[img][:, rows, 1:113], st[:],
                            AF.Sign, bias=eps_pos[:], scale=1.0,
                        )
                    nc.vector.tensor_tensor(
                        xqs[img][:, rows, 1:113],
                        xqr[img][:, rows, 1:113],
                        xqi[img][:, rows, 1:113],
                        op=ALU.add,
                    )

            def conv_image(img):
                for t in range(NTILES):
                    base = (4 * t + 2) * RS
                    pk = {}
                    for kind in ("r", "i", "s"):
                        w3 = {"r": wq_r, "i": wq_i, "s": wq_s}[kind]
                        xf = flat[(kind, img)]
                        ps = psum_pool.tile([CPB, NT], F32, tag="ps",
                                            name=f"ps_{kind}{img}_{t}")
                        pk[kind] = ps
                        part = [list(xf.ap)[0][0], CPB]
                        for p in range(4):
                            o0, o1 = TAP_OFF[2 * p], TAP_OFF[2 * p + 1]
                            rhs = bass.AP(
                                xf.tensor, xf.offset + o0 + base,
                                [part, [o1 - o0, 2], [1, NT]],
                            )
                            nc.tensor.matmul(
                                ps[:], w3[:, 2 * p:2 * p + 2, :], rhs,
                                start=(p == 0), stop=False, perf_mode=DRM,
                            )
                        nc.tensor.matmul(
                            ps[:], w3[:, 8, :],
                            xf[:, base + TAP_OFF[8]:base + TAP_OFF[8] + NT],
                            start=False, stop=True,
                        )

                    A, Bp, C = pk["r"], pk["i"], pk["s"]
                    # out_real = A - B + bias_r ; out_imag = C - A - B + bias_i
                    # DVE may read at most one PSUM operand per op, so ScalarE
                    # (fast PSUM port) evacuates negated B and A with the bias
                    # folded in:  Bn = -B + bias_r ; An = -A + (bias_i-bias_r)
                    Bn = tmp_pool.tile([CPB, NT], F32, tag="Bn")
                    nc.scalar.activation(Bn[:], Bp[:], AF.Identity,
                                         bias=bias_sb[:, 0:1], scale=-1.0)
                    An = tmp_pool.tile([CPB, NT], F32, tag="An")
                    nc.scalar.activation(An[:], A[:], AF.Identity,
                                         bias=bias_d[:], scale=-1.0)

                    Av = A[:].rearrange("p (r c) -> p r c", c=RS)
                    Cv = C[:].rearrange("p (r c) -> p r c", c=RS)
                    Anv = An[:].rearrange("p (r c) -> p r c", c=RS)
                    Bnv = Bn[:].rearrange("p (r c) -> p r c", c=RS)

                    # out_real = A + Bn  (compact: skip pad columns)
                    osb_r = out_pool.tile([CPB, TROWS, W], F32, tag="osb",
                                          name=f"osbr{img}_{t}")
                    nc.vector.tensor_add(osb_r[:], Av[:, :, 1:113], Bnv[:, :, 1:113])
                    # out_imag = (C + An) + Bn
                    t5 = tmp_pool.tile([CPB, TROWS, W], F32, tag="t5")
                    nc.vector.tensor_add(t5[:], Cv[:, :, 1:113], Anv[:, :, 1:113])
                    osb_i = out_pool.tile([CPB, TROWS, W], F32, tag="osb",
                                          name=f"osbi{img}_{t}")
                    nc.vector.tensor_add(osb_i[:], t5[:], Bnv[:, :, 1:113])

                    for coblk, osb in ((0, osb_r), (1, osb_i)):
                        ch0 = img * 2 * CPB + coblk * CPB
                        nc.sync.dma_start(
                            out_flat[ch0:ch0 + CPB, 4 * t:4 * t + TROWS, :], osb[:]
                        )

            binarize_image(0)
            conv_image(0)
            binarize_image(1)
            conv_image(1)

    _split_multiwait(nc)
    return nc


def _prep(x, weight_real, weight_imag, bias):
    x = np.ascontiguousarray(np.asarray(x, dtype=np.float32))
    wr = np.asarray(weight_real, dtype=np.float32)
    wi = np.asarray(weight_imag, dtype=np.float32)
    bias = np.asarray(bias, dtype=np.float32)
    wrT = np.ascontiguousarray(wr.transpose(1, 2, 3, 0).reshape(CPB, 9 * CPB))
    wiT = np.ascontiguousarray(wi.transpose(1, 2, 3, 0).reshape(CPB, 9 * CPB))
    bias2 = np.ascontiguousarray(bias.reshape(2, CPB).T)
    return [
        {"x": x[IMGS * c:IMGS * (c + 1)], "wrT": wrT, "wiT": wiT, "bias2": bias2}
        for c in range(N_CORES)
    ]


def kernel(x, weight_real, weight_imag, bias):
    in_maps = _prep(x, weight_real, weight_imag, bias)
    nc = build_nc()
    res = run_bass_kernel_spmd(nc, in_maps, core_ids=list(range(N_CORES)))
    return np.concatenate([res.results[i]["out"] for i in range(N_CORES)], axis=0)


def run_traced(x, weight_real, weight_imag, bias, **trace_kwargs):
    """test.py entry: same as kernel() but with neuron-profile tracing."""
    in_maps = _prep(x, weight_real, weight_imag, bias)
    nc = build_nc()
    res = run_bass_kernel_spmd(
        nc, in_maps, core_ids=list(range(N_CORES)), trace=True, **trace_kwargs
    )
    out = np.concatenate([res.results[i]["out"] for i in range(N_CORES)], axis=0)
    return out, res
